# revision 1
# baseline (speedup 1.0000x reference)
"""Trainium2 Bass kernel for nn_AttentionTIE (TIE-style edge-LayerNorm attention).

Sharding: 8 cores = (batch b = core//2) x (receiver-row half = core%2).
Each core computes the full v_sender for its batch, attention for its 1536
receiver rows, and the three projected outputs for those rows.

Algorithm per core (all shapes [partition, free]):
  v_sT  = W_send @ xT + W_mem @ sendT + res_sT            [C, N]
  v_rT  = W_recv @ xT_own + W_mem @ recvT_own + res_rT    [C, No]
  qT    = (W_q*scale) @ xT_own                            [C, No]
  std2[i,j] = u_i + w_j + (2/D) v_r.v_s - 2 m_r m_s + eps  (aug rank-2 matmul,
              u_i via ACT sqrt bias)
  score[i,j] = q.v_s + (alpha_i) - sumq_i m_s_j - M*mask   (aug rank-2 matmul +
              identity x maskbias matmul)
  T = 1/sqrt(std2); P = exp(score*T) (row denom via ACT accum)
  PT = P*T (row sum A via fused reduce); PT^T via PE transpose
  pv = PT @ [v_s | 1 | m_s]  -> out = (pv + A*v_r - (m_r A + MS)) / denom
  outputs: W'_proj @ out^T + b', W_r @ v_rT + r_b, W_s @ v_sT_own + s_b
"""
import os
import sys
from contextlib import ExitStack

import numpy as np

sys.path.insert(0, "/opt/trn_rl_repo")

import ml_dtypes  # noqa: E402
import concourse.bass as bass  # noqa: E402
import concourse.tile as tile  # noqa: E402
from concourse import bacc  # noqa: E402
from concourse import mybir  # noqa: E402
from concourse.bass_utils import run_bass_kernel_spmd  # noqa: E402

N, B, C = 3072, 4, 128
NO = N // 2          # own receiver rows per core
ITI = NO // 128      # 12 i-tiles
JCH = N // 512       # 6 j-chunks
JT = N // 128        # 24 j-tiles
EPS = 1e-5
SCALE = C ** -0.5
MASKM = 60.0         # masked-score bias: exp((score-M)*T) <= ~1e-14, ACT-range safe

F32 = mybir.dt.float32
BF16 = mybir.dt.bfloat16
U8 = mybir.dt.uint8
AF = mybir.ActivationFunctionType
ALU = mybir.AluOpType
AX = mybir.AxisListType

_CACHE = {}


def _build_program():
    nc = bacc.Bacc("TRN2", target_bir_lowering=False, debug=False, num_devices=8)

    def din(name, shape, dtype=F32):
        return nc.dram_tensor(name, list(shape), dtype, kind="ExternalInput").ap()

    def dout(name, shape, dtype=F32):
        return nc.dram_tensor(name, list(shape), dtype, kind="ExternalOutput").ap()

    xT_d = din("xT", [C, N])
    xTo_d = din("xTo", [C, NO])
    sendT_d = din("sendT", [C, N])
    sendTo_d = din("sendTo", [C, NO])
    res_sT_d = din("res_sT", [C, N])
    res_sTo_d = din("res_sTo", [C, NO])
    recvTo_d = din("recvTo", [C, NO])
    res_rTo_d = din("res_rTo", [C, NO])
    mask_d = din("mask", [NO, N], U8)
    w_send_d = din("w_send", [C, C])
    w_mem_d = din("w_mem", [C, C])
    w_recv_d = din("w_recv", [C, C])
    w_qs_d = din("w_qs", [C, C])
    w_proj_d = din("w_proj", [C, C])
    w_r_d = din("w_r", [C, C])
    w_s_d = din("w_s", [C, C])
    bp_d = din("bp", [C, 1])
    br_d = din("br", [C, 1])
    bs_d = din("bs", [C, 1])
    idf_d = din("idf", [C, C])
    idb_d = din("idb", [C, C], BF16)

    scr_mr_d = nc.dram_tensor("scr_mr", [1, NO], F32).ap()
    scr_ue_d = nc.dram_tensor("scr_ue", [1, NO], F32).ap()
    scr_ms_d = nc.dram_tensor("scr_ms", [1, N], F32).ap()
    outT_d = dout("outT", [C, NO])
    vr2T_d = dout("vr2T", [C, NO])
    vs2T_d = dout("vs2T", [C, NO])

    with tile.TileContext(nc) as tc, ExitStack() as ctx:
        const = ctx.enter_context(tc.tile_pool(name="const", bufs=1))
        per = ctx.enter_context(tc.tile_pool(name="per", bufs=1))
        stat = ctx.enter_context(tc.tile_pool(name="stat", bufs=1))
        stmp = ctx.enter_context(tc.tile_pool(name="stmp", bufs=2))
        ck = ctx.enter_context(tc.tile_pool(name="ck", bufs=2))
        strm = ctx.enter_context(tc.tile_pool(name="strm", bufs=6))
        mpool = ctx.enter_context(tc.tile_pool(name="mask", bufs=2))
        ps_mm = ctx.enter_context(tc.tile_pool(name="ps_mm", bufs=4, space="PSUM"))
        ps_tp = ctx.enter_context(tc.tile_pool(name="ps_tp", bufs=2, space="PSUM"))
        ps_pv = ctx.enter_context(tc.tile_pool(name="ps_pv", bufs=1, space="PSUM"))

        # ---------------- constants ----------------
        def cload(name, d_ap, shape, dtype=F32):
            t = const.tile(shape, dtype, tag=name)
            nc.sync.dma_start(t[:], d_ap)
            return t

        w_send = cload("w_send", w_send_d, [C, C])
        w_mem = cload("w_mem", w_mem_d, [C, C])
        w_recv = cload("w_recv", w_recv_d, [C, C])
        w_qs = cload("w_qs", w_qs_d, [C, C])
        w_proj = cload("w_proj", w_proj_d, [C, C])
        w_r = cload("w_r", w_r_d, [C, C])
        w_s = cload("w_s", w_s_d, [C, C])
        bp = cload("bp", bp_d, [C, 1])
        br_c = cload("br_c", br_d, [C, 1])
        bs_c = cload("bs_c", bs_d, [C, 1])
        idf = cload("idf", idf_d, [C, C])
        idb = cload("idb", idb_d, [C, C], BF16)

        oneD = const.tile([C, 1], F32)
        nc.gpsimd.memset(oneD[:], 1.0 / C)
        one = const.tile([C, 1], F32)
        nc.gpsimd.memset(one[:], 1.0)
        eps1 = const.tile([1, 1], F32)
        nc.gpsimd.memset(eps1[:], EPS)

        # persistent tensors
        v_sT = per.tile([C, N], F32)
        v_sTo = per.tile([C, NO], F32)
        v_rT = per.tile([C, NO], F32)
        qT = per.tile([C, NO], F32)
        vr_s = per.tile([C, NO], F32)          # v_rT * 2/C (cross lhsT)
        v_r_nat = per.tile([C, ITI * C], F32)
        v_s_aug = per.tile([C, JT * (C + 2)], BF16)
        outT_pre = per.tile([C, NO], F32)

        aug1_rhs = stat.tile([2, N], F32)      # row0 = m_s, row1 = w_j
        aug2_rhs = stat.tile([2, N], F32)      # row0 = -m_s, row1 = 1
        aug1_lhsT = stat.tile([2, NO], F32)    # row0 = -2 m_r, row1 = 1
        aug2_lhsT = stat.tile([2, NO], F32)    # row0 = sumq, row1 = alpha
        m_r_row = stat.tile([1, NO], F32)
        u_eps_row = stat.tile([1, NO], F32)
        w_row = stat.tile([1, N], F32)
        alpha_row = stat.tile([1, NO], F32)
        m_r_cols = stat.tile([C, ITI], F32)
        u_eps_cols = stat.tile([C, ITI], F32)
        m_s_cols = stat.tile([C, JT], F32)

        nc.vector.memset(aug2_rhs[:, :], 1.0)  # row0 overwritten with -m_s below
        nc.vector.memset(aug1_lhsT[:, :], 1.0)  # row0 overwritten with -2*m_r below

        def stream(d_ap, sl):
            t = strm.tile([C, 512], F32, tag="instream")
            nc.sync.dma_start(t[:], d_ap[:, sl])
            return t

        # -------- phase 1: value tensors (inputs streamed chunk-wise) --------
        for jc in range(JCH):
            sl = bass.ts(jc, 512)
            xc = stream(xT_d, sl)
            sc = stream(sendT_d, sl)
            rc = stream(res_sT_d, sl)
            ps = ps_mm.tile([C, 512], F32, tag="mm")
            nc.tensor.matmul(ps[:], w_send[:], xc[:], start=True, stop=False)
            nc.tensor.matmul(ps[:], w_mem[:], sc[:], start=False, stop=True)
            nc.vector.tensor_tensor(out=v_sT[:, sl], in0=ps[:], in1=rc[:], op=ALU.add)
        for c3 in range(NO // 512):
            sl = bass.ts(c3, 512)
            xc = stream(xTo_d, sl)
            sc = stream(sendTo_d, sl)
            rc = stream(res_sTo_d, sl)
            ps = ps_mm.tile([C, 512], F32, tag="mm")
            nc.tensor.matmul(ps[:], w_send[:], xc[:], start=True, stop=False)
            nc.tensor.matmul(ps[:], w_mem[:], sc[:], start=False, stop=True)
            nc.vector.tensor_tensor(out=v_sTo[:, sl], in0=ps[:], in1=rc[:], op=ALU.add)
            rcv = stream(recvTo_d, sl)
            rrc = stream(res_rTo_d, sl)
            ps2 = ps_mm.tile([C, 512], F32, tag="mm")
            nc.tensor.matmul(ps2[:], w_recv[:], xc[:], start=True, stop=False)
            nc.tensor.matmul(ps2[:], w_mem[:], rcv[:], start=False, stop=True)
            nc.vector.tensor_tensor(out=v_rT[:, sl], in0=ps2[:], in1=rrc[:], op=ALU.add)
            ps3 = ps_mm.tile([C, 512], F32, tag="mm")
            nc.tensor.matmul(ps3[:], w_qs[:], xc[:], start=True, stop=True)
            nc.scalar.copy(qT[:, sl], ps3[:])

        # -------- phase 2: stats --------
        # sender-side stats into aug rows
        for jc in range(JCH):
            sl = bass.ts(jc, 512)
            psm = ps_pv.tile([1, 512], F32, tag="row")
            nc.tensor.matmul(psm[:], oneD[:], v_sT[:, sl], start=True, stop=True)
            nc.scalar.copy(aug1_rhs[0:1, sl], psm[:])          # m_s
            sqc = ck.tile([C, 512], F32, tag="sqc")
            nc.scalar.activation(sqc[:], v_sT[:, sl], AF.Square)
            psq = ps_pv.tile([1, 512], F32, tag="row")
            nc.tensor.matmul(psq[:], oneD[:], sqc[:], start=True, stop=True)
            trow = stmp.tile([1, 512], F32, tag="trow")
            nc.vector.tensor_tensor(out=trow[:], in0=aug1_rhs[0:1, sl], in1=aug1_rhs[0:1, sl], op=ALU.mult)
            nc.vector.tensor_tensor(out=w_row[:, sl], in0=psq[:], in1=trow[:], op=ALU.subtract)  # w_j
            nc.scalar.mul(aug2_rhs[0:1, sl], aug1_rhs[0:1, sl], -1.0)  # -m_s

        # receiver-side stats
        for c3 in range(NO // 512):
            sl = bass.ts(c3, 512)
            psm = ps_pv.tile([1, 512], F32, tag="row")
            nc.tensor.matmul(psm[:], oneD[:], v_rT[:, sl], start=True, stop=True)
            nc.scalar.copy(m_r_row[:, sl], psm[:])
            sqc = ck.tile([C, 512], F32, tag="sqc")
            nc.scalar.activation(sqc[:], v_rT[:, sl], AF.Square)
            psq = ps_pv.tile([1, 512], F32, tag="row")
            nc.tensor.matmul(psq[:], oneD[:], sqc[:], start=True, stop=True)
            nc.scalar.activation(u_eps_row[:, sl], psq[:], AF.Identity, bias=eps1[:])
            trow = stmp.tile([1, 512], F32, tag="trow")
            nc.vector.tensor_tensor(out=trow[:], in0=m_r_row[:, sl], in1=m_r_row[:, sl], op=ALU.mult)
            nc.vector.tensor_tensor(out=u_eps_row[:, sl], in0=u_eps_row[:, sl], in1=trow[:], op=ALU.subtract)
            # sumq
            pss = ps_pv.tile([1, 512], F32, tag="row")
            nc.tensor.matmul(pss[:], one[:], qT[:, sl], start=True, stop=True)
            nc.scalar.copy(aug2_lhsT[0:1, sl], pss[:])
            # alpha = sum(q*v_r) - sumq*m_r
            qv = ck.tile([C, 512], F32, tag="sqc")
            nc.vector.tensor_tensor(out=qv[:], in0=qT[:, sl], in1=v_rT[:, sl], op=ALU.mult)
            psa = ps_pv.tile([1, 512], F32, tag="row")
            nc.tensor.matmul(psa[:], one[:], qv[:], start=True, stop=True)
            trow2 = stmp.tile([1, 512], F32, tag="trow")
            nc.vector.tensor_tensor(out=trow2[:], in0=aug2_lhsT[0:1, sl], in1=m_r_row[:, sl], op=ALU.mult)
            nc.vector.tensor_tensor(out=alpha_row[:, sl], in0=psa[:], in1=trow2[:], op=ALU.subtract)

        nc.scalar.mul(aug1_lhsT[0:1, :], m_r_row[:], -2.0)
        nc.sync.dma_start(aug1_rhs[1:2, :], w_row[:])
        nc.sync.dma_start(aug2_lhsT[1:2, :], alpha_row[:])

        # row -> column layouts via DRAM round-trip (element (p,t) = row[t*128+p])
        nc.sync.dma_start(scr_mr_d, m_r_row[:])
        nc.sync.dma_start(m_r_cols[:], scr_mr_d.rearrange("o (t p) -> (o p) t", p=128))
        nc.sync.dma_start(scr_ue_d, u_eps_row[:])
        nc.sync.dma_start(u_eps_cols[:], scr_ue_d.rearrange("o (t p) -> (o p) t", p=128))
        nc.sync.dma_start(scr_ms_d, aug1_rhs[0:1, :])
        nc.sync.dma_start(m_s_cols[:], scr_ms_d.rearrange("o (t p) -> (o p) t", p=128))

        # v_s natural (bf16, augmented) + v_r natural + scaled v_r
        v_s_aug_r = v_s_aug[:].rearrange("p (t c) -> p t c", c=C + 2)
        for g in range(JT // 4):
            pst = ps_tp.tile([C, 512], F32, tag="tp")
            for t in range(4):
                jt = g * 4 + t
                nc.tensor.transpose(pst[:, bass.ts(t, 128)], v_sT[:, bass.ts(jt, 128)], idf[:])
            src = pst[:].rearrange("p (t c) -> p t c", c=C)
            nc.scalar.copy(v_s_aug_r[:, g * 4:(g + 1) * 4, 0:C], src)
        nc.gpsimd.memset(v_s_aug_r[:, :, C:C + 1], 1.0)
        m_s_cols_r = m_s_cols[:].rearrange("p (t o) -> p t o", o=1)
        nc.scalar.copy(v_s_aug_r[:, :, C + 1:C + 2], m_s_cols_r)

        for g in range(ITI // 4):
            pst = ps_tp.tile([C, 512], F32, tag="tp")
            for t in range(4):
                it = g * 4 + t
                nc.tensor.transpose(pst[:, bass.ts(t, 128)], v_rT[:, bass.ts(it, 128)], idf[:])
            nc.scalar.copy(v_r_nat[:, bass.ts(g, 512)], pst[:])
        nc.scalar.mul(vr_s[:], v_rT[:], 2.0 / C)

        # -------- phase 3: main attention loop --------
        for it in range(ITI):
            isl = bass.ts(it, 128)
            mk8 = mpool.tile([C, N], U8, tag="mk8")
            nc.sync.dma_start(mk8[:], mask_d[bass.ts(it, 128), :])
            mkb = mpool.tile([C, N], BF16, tag="mkb")
            nc.gpsimd.tensor_scalar_mul(mkb[:], mk8[:], -MASKM)

            den_part = stmp.tile([C, 8], F32, tag="den_part")
            pv = ps_pv.tile([C, C + 2], F32)

            for jc in range(JCH):
                jsl = bass.ts(jc, 512)
                ps_v = ps_mm.tile([C, 512], F32, tag="mm")
                nc.tensor.matmul(ps_v[:], vr_s[:, isl], v_sT[:, jsl], start=True, stop=False)
                nc.tensor.matmul(ps_v[:], aug1_lhsT[:, isl], aug1_rhs[:, jsl], start=False, stop=True)
                ps_s = ps_mm.tile([C, 512], F32, tag="mm")
                nc.tensor.matmul(ps_s[:], qT[:, isl], v_sT[:, jsl], start=True, stop=False)
                nc.tensor.matmul(ps_s[:], aug2_lhsT[:, isl], aug2_rhs[:, jsl], start=False, stop=False)
                nc.tensor.matmul(ps_s[:], idb[:], mkb[:, jsl], start=False, stop=True)

                stdc = ck.tile([C, 512], F32, tag="stdc")
                nc.scalar.activation(stdc[:], ps_v[:], AF.Sqrt, bias=u_eps_cols[:, it:it + 1])
                tcc = ck.tile([C, 512], F32, tag="tcc")
                nc.vector.reciprocal_approx_fast(out=tcc[:], in_=stdc[:])
                uc = ck.tile([C, 512], F32, tag="uc")
                nc.vector.tensor_tensor(out=uc[:], in0=ps_s[:], in1=tcc[:], op=ALU.mult)
                pc = ck.tile([C, 512], F32, tag="pc")
                nc.scalar.activation(pc[:], uc[:], AF.Exp, accum_out=den_part[:, jc:jc + 1])
                ptc = ck.tile([C, 512], F32, tag="ptc")
                nc.vector.tensor_tensor(out=ptc[:], in0=pc[:], in1=tcc[:], op=ALU.mult)

                pst = ps_tp.tile([C, 512], F32, tag="tp")
                for t in range(4):
                    nc.tensor.transpose(pst[:, bass.ts(t, 128)], ptc[:, bass.ts(t, 128)], idf[:])
                pttc = ck.tile([C, 512], BF16, tag="pttc")
                nc.scalar.copy(pttc[:], pst[:])
                for t in range(4):
                    jt = jc * 4 + t
                    nc.tensor.matmul(
                        pv[:], pttc[:, bass.ts(t, 128)], v_s_aug_r[:, jt, :],
                        start=(jc == 0 and t == 0), stop=(jc == JCH - 1 and t == 3))

            den = stmp.tile([C, 1], F32, tag="den")
            nc.vector.tensor_reduce(den[:], den_part[:, 0:JCH], axis=AX.X, op=ALU.add)
            rcol = stmp.tile([C, 1], F32, tag="rcol")
            nc.vector.reciprocal(rcol[:], den[:])
            ams = stmp.tile([C, 2], F32, tag="ams")
            nc.scalar.copy(ams[:], pv[:, C:C + 2])
            t1 = stmp.tile([C, 1], F32, tag="t1")
            nc.vector.scalar_tensor_tensor(
                out=t1[:], in0=ams[:, 0:1], scalar=m_r_cols[:, it:it + 1], in1=ams[:, 1:2],
                op0=ALU.mult, op1=ALU.add)
            brr = stmp.tile([C, 1], F32, tag="brr")
            nc.vector.scalar_tensor_tensor(
                out=brr[:], in0=t1[:], scalar=-1.0, in1=rcol[:], op0=ALU.mult, op1=ALU.mult)
            x1 = stmp.tile([C, C], F32, tag="x1")
            nc.vector.scalar_tensor_tensor(
                out=x1[:], in0=v_r_nat[:, isl], scalar=ams[:, 0:1], in1=pv[:, 0:C],
                op0=ALU.mult, op1=ALU.add)
            x2 = stmp.tile([C, C], F32, tag="x2")
            nc.scalar.activation(x2[:], x1[:], AF.Identity, bias=brr[:], scale=rcol[:])
            pso = ps_tp.tile([C, C], F32, tag="tp")
            nc.tensor.transpose(pso[:], x2[:], idf[:])
            nc.scalar.copy(outT_pre[:, isl], pso[:])

        # -------- phase 4: output projections --------
        for w, bias_col, rhs, out_d in (
            (w_proj, bp, outT_pre, outT_d),
            (w_r, br_c, v_rT, vr2T_d),
            (w_s, bs_c, v_sTo, vs2T_d),
        ):
            for c3 in range(NO // 512):
                sl = bass.ts(c3, 512)
                ps = ps_mm.tile([C, 512], F32, tag="mm")
                nc.tensor.matmul(ps[:], w[:], rhs[:, sl], start=True, stop=True)
                ob = stmp.tile([C, 512], F32, tag="ob")
                nc.scalar.activation(ob[:], ps[:], AF.Identity, bias=bias_col[:])
                nc.sync.dma_start(out_d[:, sl], ob[:])

    nc.compile()
    return nc


def _host_prep(inputs):
    """Returns (in_maps list of 8 dicts, misc)"""
    f32 = np.float32
    x = np.ascontiguousarray(np.asarray(inputs["x"], f32))
    recv = np.asarray(inputs["receiver_val_res"], f32)
    send = np.asarray(inputs["sender_val_res"], f32)
    res_r = np.asarray(inputs["residual_receiver"], f32)
    res_s = np.asarray(inputs["residual_sender"], f32)
    mask = np.asarray(inputs["attn_mask"])
    ra = np.asarray(inputs["relation_attn"], f32)
    q_w = np.asarray(inputs["q_w"], f32)
    proj_w = np.asarray(inputs["proj_w"], f32)
    proj_b = np.asarray(inputs["proj_b"], f32)
    r_w = np.asarray(inputs["r_w"], f32)
    r_b = np.asarray(inputs["r_b"], f32)
    s_w = np.asarray(inputs["s_w"], f32)
    s_b = np.asarray(inputs["s_b"], f32)
    n_weight = np.asarray(inputs["n_weight"], f32)
    n_bias = np.asarray(inputs["n_bias"], f32)

    mem_w, recv_w, send_w = ra[:, :C], ra[:, C:2 * C], ra[:, 2 * C:]
    w_proj_eff = proj_w * n_weight[None, :]
    b_proj_eff = proj_w @ n_bias + proj_b

    cc = np.ascontiguousarray
    weights = {
        "w_send": cc(send_w.T), "w_mem": cc(mem_w.T), "w_recv": cc(recv_w.T),
        "w_qs": cc(q_w.T * SCALE), "w_proj": cc(w_proj_eff.T),
        "w_r": cc(r_w.T), "w_s": cc(s_w.T),
        "bp": cc(b_proj_eff[:, None]), "br": cc(r_b[:, None]), "bs": cc(s_b[:, None]),
        "idf": cc(np.eye(C, dtype=f32)),
        "idb": cc(np.eye(C).astype(ml_dtypes.bfloat16)),
    }

    in_maps = []
    for core in range(8):
        b, half = core // 2, core % 2
        i0, i1 = half * NO, (half + 1) * NO
        xb = cc(x[:, b, :].T)                      # [C, N]
        sb = cc(send[:, b, :].T)
        rsb = cc(res_s[:, b, :].T)
        m = {
            "xT": xb, "xTo": cc(xb[:, i0:i1]),
            "sendT": sb, "sendTo": cc(sb[:, i0:i1]),
            "res_sT": rsb, "res_sTo": cc(rsb[:, i0:i1]),
            "recvTo": cc(recv[i0:i1, b, :].T),
            "res_rTo": cc(res_r[i0:i1, b, :].T),
            "mask": cc(mask[b, 0, i0:i1, :].astype(np.uint8)),
        }
        m.update(weights)
        in_maps.append(m)
    return in_maps


def kernel(**inputs):
    if "nc" not in _CACHE:
        _CACHE["nc"] = _build_program()
    nc = _CACHE["nc"]
    in_maps = _host_prep(inputs)
    res = run_bass_kernel_spmd(nc, in_maps, core_ids=list(range(8)))
    out = np.zeros((N, B, C), np.float32)
    vr2 = np.zeros((N, B, C), np.float32)
    vs2 = np.zeros((N, B, C), np.float32)
    for core in range(8):
        b, half = core // 2, core % 2
        i0, i1 = half * NO, (half + 1) * NO
        r = res.results[core]
        out[i0:i1, b, :] = r["outT"].T
        vr2[i0:i1, b, :] = r["vr2T"].T
        vs2[i0:i1, b, :] = r["vs2T"].T
    return out, vr2, vs2



# revision 38
# speedup vs baseline: 3612.6248x; 3612.6248x over previous
"""Trainium2 Bass kernel for nn_AttentionTIE (TIE-style edge-LayerNorm attention).

Sharding: 8 cores = (batch b = core//2) x (receiver-row half = core%2).
Each core computes full v_sender for its batch, attention for its 1536
receiver rows, and the three projected outputs for those rows.

v2 design (vs the f32 baseline):
- All big matmuls bf16 / f32r (1 cyc/row) instead of f32 (4 cyc/row).
- Centered values c_s = v_s - m_s, c_r = v_r - m_r kill the mean rank-2
  terms: std2 = u_i + w_j + (2/D) c_r.c_s, score = q.c_s + alpha_i,
  out_pre = (PV[c_s] + A*c_r) / denom. PV aug channels = [c_s | 1].
- ACT engine runs ONLY exp-table functions (Exp/Copy/Identity/Square):
  zero activation-table reloads in steady state (baseline lost 124us/core
  to Sqrt<->Exp table thrash).
- rsqrt via one custom DVE op (quadratic minimax seed + folded Newton
  step, emitted scaled by 2/(3*sqrt(3)) to fit the 8-stage DVE pipeline;
  the 3*sqrt(3)/2 constant folds into host-side q / proj weights).
- mask host-converted to fp8e5m2 {0, -128}, injected into the score PSUM
  via an fp8 identity matmul (same DMA bytes as u8, kills the 53us gpsimd
  u8->bf16 conversion).
- PT = P*Tq runs on the otherwise-idle Pool (gpsimd) engine.
- Host rolls x/send/res_s/mask columns so the own receiver half is always
  columns [0, NO) -- one static program for all 8 cores.
"""
import sys
from contextlib import ExitStack

import numpy as np

sys.path.insert(0, "/opt/trn_rl_repo")

import ml_dtypes  # noqa: E402
import concourse.bass as bass  # noqa: E402
import concourse.tile as tile  # noqa: E402
from concourse import bacc  # noqa: E402
from concourse import mybir  # noqa: E402
from concourse.bass_utils import run_bass_kernel_spmd  # noqa: E402

# ---------------------------------------------------------------- custom DVE op
# Tq = s - x*s^3 with s = C0 + x*(C1 + x*C2): equals (2/(3*sqrt(3)))*rsqrt(x)
# for the minimax coefficients below (x in [0.85, 4.6], max rel err 2.6e-3
# after the folded Newton step). Exactly 8 ALU stages == the v3 pipeline.
import concourse.dve_ops as dve_ops  # noqa: E402
from concourse.dve_spec import C0, C1, C2, Spec, Src0, sq  # noqa: E402

RSQ_NAME = "RSQRT_SCALED_ANT"
RSQ_C = (0.7774802208033782, -0.23189214904335725, 0.02691905161330013)
RSQ_K = 3.0 * np.sqrt(3.0)  # consumers multiply this back in (host-folded)


def _rsq_ref(in0, in1, s0, s1, imm2):
    s = s0 + in0 * (s1 + in0 * imm2)
    return (s - in0 * s * s * s).astype(np.float32)


def _register_rsqrt_op():
    for op in dve_ops.OPS:
        if op.name == RSQ_NAME:
            return op
    _x = Src0
    _s = C0 + _x * (C1 + _x * C2)
    spec = Spec(body=_s - _x * (_s * sq(_s)), reference=_rsq_ref)
    row = dve_ops._CUSTOM_DVE_ROW_BASE + len(dve_ops.OPS)
    assert row < 0x20
    from concourse.dve_spec import lower as _lower
    from concourse.dve_uop import DveOpSpec

    shas = {}
    for ver in ("v3", "v4"):
        s = DveOpSpec(
            name=RSQ_NAME, opcode=row, uops=_lower(spec, ver=ver), rd1_en=False
        )
        shas[ver] = s.sha(ver)
    op = dve_ops.DveOp(RSQ_NAME, spec, subdim=False, uops_sha=shas)
    dve_ops.OPS.append(op)
    dve_ops.CUSTOM_DVE_SPECS[RSQ_NAME] = spec
    dve_ops._SUB_OPCODE_FOR_NAME[RSQ_NAME] = row
    return op


RSQ_OP = _register_rsqrt_op()

# ---------------------------------------------------------------------- consts
N, B, C = 3072, 4, 128
NO = N // 2          # own receiver rows per core
ITI = NO // 128      # 12 i-tiles
JCH = N // 512       # 6 j-chunks
JT = N // 128        # 24 j-tiles
EPS = 1e-5
SCALE = C ** -0.5
MASKM = 128.0        # fp8e5m2-exact mask bias (pre-Tq units)

F32 = mybir.dt.float32
F32R = mybir.dt.float32r
BF16 = mybir.dt.bfloat16
FP8 = mybir.dt.float8e5
AF = mybir.ActivationFunctionType
ALU = mybir.AluOpType
AX = mybir.AxisListType

_CACHE = {}


def _build_program():
    nc = bacc.Bacc("TRN2", target_bir_lowering=False, debug=False, num_devices=8)

    def din(name, shape, dtype=F32):
        return nc.dram_tensor(name, list(shape), dtype, kind="ExternalInput").ap()

    def dout(name, shape, dtype=F32):
        return nc.dram_tensor(name, list(shape), dtype, kind="ExternalOutput").ap()

    xT_d = din("xT", [C, N], F32R)
    sendT_d = din("sendT", [C, N], F32R)
    res_sT_d = din("res_sT", [C, N], F32R)
    recvTo_d = din("recvTo", [C, NO], F32R)
    res_rTo_d = din("res_rTo", [C, NO], F32R)
    mask_d = din("mask", [NO, N], FP8)
    w_send_d = din("w_send", [C, C], F32R)
    w_mem_d = din("w_mem", [C, C], F32R)
    w_recv_d = din("w_recv", [C, C], F32R)
    w_qs_d = din("w_qs", [C, C], F32R)
    w_proj_d = din("w_proj", [C, C], F32R)
    w_r_d = din("w_r", [C, C], F32R)
    w_s_d = din("w_s", [C, C], F32R)
    bp_d = din("bp", [C, 1])
    br_d = din("br", [C, 1])
    bs_d = din("bs", [C, 1])
    idf_d = din("idf", [C, C])
    idb_d = din("idb", [C, C], BF16)
    idr_d = din("idr", [C, C], F32R)
    oneD_d = din("oneD", [C, 1], F32R)
    one_d = din("one", [C, 1], F32R)
    onesr_d = din("onesr", [1, C], F32R)
    onesN_d = din("onesN", [1, N], F32R)
    id8_d = din("id8", [C, C], FP8)

    scr_ms_d = nc.dram_tensor("scr_ms", [1, N], F32).ap()
    scr_msq_d = nc.dram_tensor("scr_msq", [1, N], F32).ap()
    scr_w_d = nc.dram_tensor("scr_w", [1, N], F32).ap()
    scr_mr_d = nc.dram_tensor("scr_mr", [1, NO], F32).ap()
    scr_mrq_d = nc.dram_tensor("scr_mrq", [1, NO], F32).ap()
    scr_sq_d = nc.dram_tensor("scr_sq", [1, NO], F32).ap()
    scr_qv_d = nc.dram_tensor("scr_qv", [1, NO], F32).ap()
    scr_u_d = nc.dram_tensor("scr_u", [1, NO], F32).ap()
    outT_d = dout("outT", [C, NO])
    vr2T_d = dout("vr2T", [C, NO])
    vs2T_d = dout("vs2T", [C, NO])

    with tile.TileContext(nc) as tc, ExitStack() as ctx:
        const = ctx.enter_context(tc.tile_pool(name="const", bufs=1))
        per = ctx.enter_context(tc.tile_pool(name="per", bufs=1))
        stat = ctx.enter_context(tc.tile_pool(name="stat", bufs=1))
        stmp = ctx.enter_context(tc.tile_pool(name="stmp", bufs=2))
        ck = ctx.enter_context(tc.tile_pool(name="ck", bufs=7))
        strm = ctx.enter_context(tc.tile_pool(name="strm", bufs=6))
        mpool = ctx.enter_context(tc.tile_pool(name="mask", bufs=2))
        ps_mm = ctx.enter_context(tc.tile_pool(name="ps_mm", bufs=5, space="PSUM"))
        ps_pv = ctx.enter_context(tc.tile_pool(name="ps_pv", bufs=2, space="PSUM"))

        # ---------------- constants ----------------
        def cload(name, d_ap, shape, dtype=F32):
            t = const.tile(shape, dtype, tag=name)
            nc.sync.dma_start(t[:], d_ap)
            return t

        w_send = cload("w_send", w_send_d, [C, C], F32R)
        w_mem = cload("w_mem", w_mem_d, [C, C], F32R)
        w_recv = cload("w_recv", w_recv_d, [C, C], F32R)
        w_qs = cload("w_qs", w_qs_d, [C, C], F32R)
        w_proj = cload("w_proj", w_proj_d, [C, C], F32R)
        w_r = cload("w_r", w_r_d, [C, C], F32R)
        w_s = cload("w_s", w_s_d, [C, C], F32R)
        bp = cload("bp", bp_d, [C, 1])
        br_c = cload("br_c", br_d, [C, 1])
        bs_c = cload("bs_c", bs_d, [C, 1])
        idf = cload("idf", idf_d, [C, C])
        idb = cload("idb", idb_d, [C, C], BF16)
        id8 = cload("id8", id8_d, [C, C], FP8)
        idr = cload("idr", idr_d, [C, C], F32R)

        oneD = cload("oneD", oneD_d, [C, 1], F32R)      # 1/C col (means)
        one = cload("one", one_d, [C, 1], F32R)         # ones col (sums)
        ones_row = cload("ones_row", onesr_d, [1, C], F32R)  # bcast lhsT

        def r(ap):
            return ap.bitcast(F32R)

        # persistent tensors
        v_sT = per.tile([C, N], F32R)
        v_rT = per.tile([C, NO], F32R)
        qT = per.tile([C, NO], F32R)
        qTb = per.tile([C, NO], BF16)
        c_sT = per.tile([C, N], BF16)
        c_rTs = per.tile([C, NO], BF16)         # (v_r - m_r) * 2/C
        c_r_nat = per.tile([C, ITI * C], BF16)  # scaled c_r, natural layout
        c_s_aug = per.tile([C, JT * (C + 1)], BF16)
        outT_pre = per.tile([C, NO], F32R)

        aug_lhsT = stat.tile([2, NO], F32R)      # row0 = u_i+eps, row1 = 1
        aug_rhs = stat.tile([2, N], F32R)        # row0 = 1, row1 = w_j
        m_s_row = stat.tile([1, N], F32R)
        msq_row = stat.tile([1, N], F32)
        m_r_row = stat.tile([1, NO], F32R)
        mrq_row = stat.tile([1, NO], F32)
        sumq_row = stat.tile([1, NO], F32)
        qvr_row = stat.tile([1, NO], F32)
        alpha_cols = stat.tile([C, ITI], F32)
        m_s_cols = stat.tile([C, JT], F32)
        neg_ms_cols = stat.tile([C, JT], F32)
        msq_cols = stat.tile([C, JT], F32)
        w_cols = stat.tile([C, JT], F32R)
        m_r_cols = stat.tile([C, ITI], F32)
        mrq_cols = stat.tile([C, ITI], F32)
        sumq_cols = stat.tile([C, ITI], F32)
        u_cols = stat.tile([C, ITI], F32R)

        nc.sync.dma_start(aug_lhsT[1:2, :], onesN_d[:, 0:NO])
        nc.sync.dma_start(aug_rhs[0:1, :], onesN_d)

        def stream(d_ap, sl, tag="instream"):
            t = strm.tile([C, 512], F32R, tag=tag)
            nc.sync.dma_start(t[:], d_ap[:, sl])
            return t

        # -------- phase 1: value tensors --------
        # residuals folded into the PSUM accumulation via f32r identity
        # matmul; PSUM evacuated by ACT (keeps DVE free for the main loop).
        for jc in range(JCH):
            sl = bass.ts(jc, 512)
            xc = stream(xT_d, sl)
            sc = stream(sendT_d, sl)
            rc = stream(res_sT_d, sl)
            ps = ps_mm.tile([C, 512], F32, tag="mm")
            nc.tensor.matmul(ps[:], w_send[:], xc[:], start=True, stop=False)
            nc.tensor.matmul(ps[:], w_mem[:], sc[:], start=False, stop=False)
            nc.tensor.matmul(ps[:], idr[:], rc[:], start=False, stop=True)
            nc.scalar.copy(v_sT[:, sl], ps[:])
            # m_s and mean-square rank-1s straight off the SBUF copy
            psm_f = ps_mm.tile([C, 512], F32, tag="mm")
            nc.tensor.matmul(psm_f[0:1, :], oneD[:], v_sT[:, sl], start=True, stop=True)
            nc.vector.tensor_copy(out=m_s_row[:, sl], in_=psm_f[0:1, :])
            sqc = ck.tile([C, 512], F32R, tag="sqc")
            nc.scalar.activation(sqc[:], v_sT[:, sl], AF.Square)
            psq_f = ps_mm.tile([C, 512], F32, tag="mm")
            nc.tensor.matmul(psq_f[0:1, :], oneD[:], sqc[:], start=True, stop=True)
            nc.vector.tensor_copy(out=msq_row[:, sl], in_=psq_f[0:1, :])
            # c_sT = v_sT - bcast(m_s)
            psb = ps_mm.tile([C, 512], F32, tag="mm")
            nc.tensor.matmul(psb[:], ones_row[:], m_s_row[0:1, sl], start=True, stop=True)
            nc.vector.tensor_tensor(out=c_sT[:, sl], in0=v_sT[:, sl], in1=psb[:], op=ALU.subtract)

        for c3 in range(NO // 512):
            sl = bass.ts(c3, 512)
            xc = stream(xT_d, sl)
            rcv = stream(recvTo_d, sl)
            rrc = stream(res_rTo_d, sl)
            ps2 = ps_mm.tile([C, 512], F32, tag="mm")
            nc.tensor.matmul(ps2[:], w_recv[:], xc[:], start=True, stop=False)
            nc.tensor.matmul(ps2[:], w_mem[:], rcv[:], start=False, stop=False)
            nc.tensor.matmul(ps2[:], idr[:], rrc[:], start=False, stop=True)
            nc.scalar.copy(v_rT[:, sl], ps2[:])
            ps3 = ps_mm.tile([C, 512], F32, tag="mm")
            nc.tensor.matmul(ps3[:], w_qs[:], xc[:], start=True, stop=True)
            nc.scalar.copy(qT[:, sl], ps3[:])
            nc.scalar.copy(qTb[:, sl], ps3[:])

        # -------- phase 2: stats (row extraction, arithmetic in col space) --
        for c3 in range(NO // 512):
            sl = bass.ts(c3, 512)
            psm_f = ps_mm.tile([C, 512], F32, tag="mm")
            nc.tensor.matmul(psm_f[0:1, :], oneD[:], v_rT[:, sl], start=True, stop=True)
            nc.vector.tensor_copy(out=m_r_row[:, sl], in_=psm_f[0:1, :])
            sqc = ck.tile([C, 512], F32R, tag="sqc")
            nc.scalar.activation(sqc[:], v_rT[:, sl], AF.Square)
            psq_f = ps_mm.tile([C, 512], F32, tag="mm")
            nc.tensor.matmul(psq_f[0:1, :], oneD[:], sqc[:], start=True, stop=True)
            nc.vector.tensor_copy(out=mrq_row[:, sl], in_=psq_f[0:1, :])
            pss_f = ps_mm.tile([C, 512], F32, tag="mm")
            nc.tensor.matmul(pss_f[0:1, :], one[:], qT[:, sl], start=True, stop=True)
            nc.vector.tensor_copy(out=sumq_row[:, sl], in_=pss_f[0:1, :])
            qv = ck.tile([C, 512], F32R, tag="sqc")
            nc.vector.tensor_tensor(out=qv[:], in0=qT[:, sl], in1=v_rT[:, sl], op=ALU.mult)
            psa_f = ps_mm.tile([C, 512], F32, tag="mm")
            nc.tensor.matmul(psa_f[0:1, :], one[:], qv[:], start=True, stop=True)
            nc.vector.tensor_copy(out=qvr_row[:, sl], in_=psa_f[0:1, :])
            # c_rTs = (v_r - m_r) * 2/C  -> bf16
            psb = ps_mm.tile([C, 512], F32, tag="mm")
            nc.tensor.matmul(psb[:], ones_row[:], m_r_row[0:1, sl], start=True, stop=True)
            nc.vector.tensor_tensor(out=c_rTs[:, sl], in0=v_rT[:, sl], in1=psb[:], op=ALU.subtract)
        nc.scalar.mul(c_rTs[:], c_rTs[:], 2.0 / C)

        # rows -> column layouts via direct SBUF->SBUF strided DMA
        def to_cols(row_t, scr_d, cols_t):
            src_ap = row_t[:]
            if src_ap.dtype != scr_d.dtype:
                src_ap = src_ap.bitcast(scr_d.dtype)
            nc.sync.dma_start(scr_d, src_ap)
            nc.sync.dma_start(
                cols_t[:], scr_d.rearrange("o (t p) -> (o p) t", p=128))

        to_cols(m_s_row, scr_ms_d, m_s_cols)
        to_cols(msq_row, scr_msq_d, msq_cols)
        to_cols(m_r_row, scr_mr_d, m_r_cols)
        to_cols(mrq_row, scr_mrq_d, mrq_cols)
        to_cols(sumq_row, scr_sq_d, sumq_cols)
        to_cols(qvr_row, scr_qv_d, alpha_cols)  # raw q.v_r; corrected below

        nc.scalar.mul(neg_ms_cols[:], m_s_cols[:], -1.0)
        # w = msq - m_s^2  (per j-tile columns)
        nc.vector.tensor_tensor(out=w_cols[:], in0=m_s_cols[:], in1=m_s_cols[:], op=ALU.mult)
        nc.vector.tensor_tensor(out=w_cols[:], in0=msq_cols[:], in1=w_cols[:], op=ALU.subtract)
        # u_eps = mrq - m_r^2 + EPS
        nc.vector.tensor_tensor(out=u_cols[:], in0=m_r_cols[:], in1=m_r_cols[:], op=ALU.mult)
        nc.vector.scalar_tensor_tensor(
            out=u_cols[:], in0=mrq_cols[:], scalar=EPS, in1=u_cols[:],
            op0=ALU.add, op1=ALU.subtract)
        # alpha = q.v_r - sumq*m_r
        nc.vector.tensor_tensor(out=sumq_cols[:], in0=sumq_cols[:], in1=m_r_cols[:], op=ALU.mult)
        nc.vector.tensor_tensor(out=alpha_cols[:], in0=alpha_cols[:], in1=sumq_cols[:], op=ALU.subtract)

        # w and u_eps back to row layout for the aug matmul operands
        nc.sync.dma_start(scr_w_d.rearrange("o (t p) -> (o p) t", p=128),
                          w_cols[:].bitcast(F32))
        nc.sync.dma_start(aug_rhs[1:2, :], scr_w_d.bitcast(F32R))
        nc.sync.dma_start(scr_u_d.rearrange("o (t p) -> (o p) t", p=128),
                          u_cols[:].bitcast(F32))
        nc.sync.dma_start(aug_lhsT[0:1, :], scr_u_d.bitcast(F32R))

        # c_s natural + ones channel; center during PSUM evacuation with
        # per-partition bias = -m_s for the j-tile.
        c_s_aug_r = c_s_aug[:].rearrange("p (t c) -> p t c", c=C + 1)
        for g in range(JT // 4):
            pst = ps_mm.tile([C, 512], F32, tag="mm")
            for t in range(4):
                jt = g * 4 + t
                nc.tensor.transpose(pst[:, bass.ts(t, 128)], v_sT[:, bass.ts(jt, 128)].bitcast(F32), idf[:])
            for t in range(4):
                jt = g * 4 + t
                nc.scalar.activation(
                    c_s_aug_r[:, jt, 0:C], pst[:, bass.ts(t, 128)], AF.Identity,
                    bias=neg_ms_cols[:, jt:jt + 1])
        nc.gpsimd.memset(c_s_aug_r[:, :, C:C + 1], 1.0)

        # c_r natural (scaled): transpose c_rTs (bf16, 1 cyc/row)
        neg_mr2_cols = stat.tile([C, ITI], F32)
        nc.scalar.mul(neg_mr2_cols[:], m_r_cols[:], -2.0 / C)
        for g in range(ITI // 4):
            pst = ps_mm.tile([C, 512], F32, tag="mm")
            for t in range(4):
                it = g * 4 + t
                nc.tensor.transpose(pst[:, bass.ts(t, 128)], v_rT[:, bass.ts(it, 128)].bitcast(F32), idf[:])
            for t in range(4):
                it = g * 4 + t
                nc.scalar.activation(
                    c_r_nat[:, bass.ts(it, 128)], pst[:, bass.ts(t, 128)], AF.Identity,
                    bias=neg_mr2_cols[:, it:it + 1], scale=2.0 / C)

        # vr2 / vs2 projections don't depend on attention: emit them here so
        # they overlap the main loop (outT projection stays at the end).
        for w, bias_col, rhs_t, out_d in (
            (w_r, br_c, v_rT, vr2T_d),
            (w_s, bs_c, v_sT, vs2T_d),
        ):
            for c3 in range(NO // 512):
                sl = bass.ts(c3, 512)
                ps = ps_mm.tile([C, 512], F32, tag="mm")
                nc.tensor.matmul(ps[:], w[:], rhs_t[:, sl], start=True, stop=True)
                ob = stmp.tile([C, 512], F32, tag="ob")
                nc.scalar.activation(ob[:], ps[:], AF.Identity, bias=bias_col[:])
                nc.sync.dma_start(out_d[:, sl], ob[:])

        # -------- phase 3: main attention loop (flat software pipeline) ----
        # Per slot (it, jc): emit MM + elementwise chain; the transpose + PV
        # stage for slot k-LAG runs behind so the PE's in-order queue never
        # head-of-line blocks on the elementwise chain. PT transposes go
        # through the DMA XBAR (idle engine) instead of PE+ACT evacuation.
        rsq = RSQ_C
        LAG = 4
        EPI = 3
        slots = [(it, jc) for it in range(ITI) for jc in range(JCH)]
        mk8_tiles = {}
        den_parts = {}
        pv_tiles = {}
        pv_sbs = {}
        ptc_tiles = {}

        def prefetch_mask(it):
            mk8 = mpool.tile([C, N], FP8, tag="mk8")
            nc.sync.dma_start(mk8[:], mask_d[bass.ts(it, 128), :])
            mk8_tiles[it] = mk8

        def emit_front(it, jc):
            isl = bass.ts(it, 128)
            jsl = bass.ts(jc, 512)
            if jc == 0:
                den_parts[it] = stmp.tile([C, 8], F32, tag="den_part", name=f"denp{it}")
                pv_tiles[it] = ps_pv.tile([C, C + 1], F32, tag="pv", name=f"pv{it}")
                if it + 1 < ITI:
                    prefetch_mask(it + 1)
            ps_v = ps_mm.tile([C, 512], F32, tag="mm")
            nc.tensor.matmul(ps_v[:], c_rTs[:, isl], c_sT[:, jsl], start=True, stop=False)
            nc.tensor.matmul(ps_v[:], aug_lhsT[:, isl], aug_rhs[:, jsl], start=False, stop=True)
            ps_s = ps_mm.tile([C, 512], F32, tag="mm")
            nc.tensor.matmul(ps_s[:], qTb[:, isl], c_sT[:, jsl], start=True, stop=False)
            nc.tensor.matmul(ps_s[:], id8[:], mk8_tiles[it][:, jsl], start=False, stop=True)

            tq = ck.tile([C, 512], F32, tag="tq")
            nc.vector._custom_dve(
                RSQ_OP, out=tq[:], in0=ps_v[:],
                s0=rsq[0], s1=rsq[1], imm2=rsq[2])
            uc = ck.tile([C, 512], BF16, tag="uc")
            nc.vector.scalar_tensor_tensor(
                out=uc[:], in0=ps_s[:], scalar=alpha_cols[:, it:it + 1],
                in1=tq[:], op0=ALU.add, op1=ALU.mult)
            pc = ck.tile([C, 512], BF16, tag="pc")
            nc.scalar.activation(pc[:], uc[:], AF.Exp, accum_out=den_parts[it][:, jc:jc + 1])
            ptc = ck.tile([C, 512], BF16, tag="ptc")
            nc.gpsimd.tensor_tensor(out=ptc[:], in0=pc[:], in1=tq[:], op=ALU.mult)
            ptc_tiles[(it, jc)] = ptc

        def emit_back(it, jc):
            ptc = ptc_tiles.pop((it, jc))
            ptt = ck.tile([C, 512], BF16, tag="ptt")
            nc.sync.dma_start_transpose(
                ptt[:].rearrange("p (t c) -> p t c", c=128), ptc[:])
            pv = pv_tiles[it]
            for t in range(4):
                jt = jc * 4 + t
                nc.tensor.matmul(
                    pv[:], ptt[:, bass.ts(t, 128)], c_s_aug_r[:, jt, :],
                    start=(jc == 0 and t == 0), stop=(jc == JCH - 1 and t == 3))


        def emit_epilogue(it):
            isl = bass.ts(it, 128)
            pv_sb = pv_tiles.pop(it)
            den_part = den_parts.pop(it)
            den = stmp.tile([C, 1], F32, tag="den")
            nc.vector.tensor_reduce(den[:], den_part[:, 0:JCH], axis=AX.X, op=ALU.add)
            rcol = stmp.tile([C, 1], F32, tag="rcol")
            nc.vector.reciprocal_approx_fast(out=rcol[:], in_=den[:])
            a2 = stmp.tile([C, 1], F32, tag="a2")
            nc.scalar.mul(a2[:], pv_sb[:, C:C + 1], float(C) / 2.0)
            x1 = stmp.tile([C, C], F32, tag="x1")
            nc.vector.scalar_tensor_tensor(
                out=x1[:], in0=c_r_nat[:, isl], scalar=a2[:], in1=pv_sb[:, 0:C],
                op0=ALU.mult, op1=ALU.add)
            x2 = stmp.tile([C, C], F32, tag="x2")
            nc.scalar.mul(x2[:], x1[:], rcol[:])
            pso = ps_mm.tile([C, 512], F32, tag="mm")
            nc.tensor.transpose(pso[:, 0:C], x2[:], idf[:])
            nc.scalar.copy(outT_pre[:, isl], pso[:, 0:C])

        prefetch_mask(0)
        for k in range(len(slots) + LAG + EPI):
            if k < len(slots):
                emit_front(*slots[k])
            kb = k - LAG
            if 0 <= kb < len(slots):
                emit_back(*slots[kb])
            ke = k - LAG - EPI
            if ke >= 0 and slots[ke][1] == JCH - 1:
                emit_epilogue(slots[ke][0])

        # -------- phase 4: attention output projection --------
        for c3 in range(NO // 512):
            sl = bass.ts(c3, 512)
            ps = ps_mm.tile([C, 512], F32, tag="mm")
            nc.tensor.matmul(ps[:], w_proj[:], outT_pre[:, sl], start=True, stop=True)
            ob = stmp.tile([C, 512], F32, tag="ob")
            nc.scalar.activation(ob[:], ps[:], AF.Identity, bias=bp[:])
            nc.sync.dma_start(outT_d[:, sl], ob[:])

    nc.compile()
    return nc


def _host_prep(inputs):
    """Returns in_maps list of 8 dicts."""
    f32 = np.float32
    x = np.ascontiguousarray(np.asarray(inputs["x"], f32))
    recv = np.asarray(inputs["receiver_val_res"], f32)
    send = np.asarray(inputs["sender_val_res"], f32)
    res_r = np.asarray(inputs["residual_receiver"], f32)
    res_s = np.asarray(inputs["residual_sender"], f32)
    mask = np.asarray(inputs["attn_mask"])
    ra = np.asarray(inputs["relation_attn"], f32)
    q_w = np.asarray(inputs["q_w"], f32)
    proj_w = np.asarray(inputs["proj_w"], f32)
    proj_b = np.asarray(inputs["proj_b"], f32)
    r_w = np.asarray(inputs["r_w"], f32)
    r_b = np.asarray(inputs["r_b"], f32)
    s_w = np.asarray(inputs["s_w"], f32)
    s_b = np.asarray(inputs["s_b"], f32)
    n_weight = np.asarray(inputs["n_weight"], f32)
    n_bias = np.asarray(inputs["n_bias"], f32)

    mem_w, recv_w, send_w = ra[:, :C], ra[:, C:2 * C], ra[:, 2 * C:]
    w_proj_eff = proj_w * n_weight[None, :] * (RSQ_K / 2.0)
    b_proj_eff = proj_w @ n_bias + proj_b
    q_scale = SCALE * RSQ_K / 2.0

    cc = np.ascontiguousarray
    weights = {
        "w_send": cc(send_w.T), "w_mem": cc(mem_w.T), "w_recv": cc(recv_w.T),
        "w_qs": cc(q_w.T * q_scale),
        "w_proj": cc(w_proj_eff.T),
        "w_r": cc(r_w.T), "w_s": cc(s_w.T),
        "bp": cc(b_proj_eff[:, None]), "br": cc(r_b[:, None]), "bs": cc(s_b[:, None]),
        "idf": cc(np.eye(C, dtype=f32)),
        "idr": cc(np.eye(C, dtype=f32)),
        "oneD": cc(np.full((C, 1), 1.0 / C, f32)),
        "one": cc(np.ones((C, 1), f32)),
        "onesr": cc(np.ones((1, C), f32)),
        "onesN": cc(np.ones((1, N), f32)),
        "idb": cc(np.eye(C).astype(ml_dtypes.bfloat16)),
        "id8": cc(np.eye(C).astype(ml_dtypes.float8_e5m2)),
    }

    in_maps = []
    for core in range(8):
        b, half = core // 2, core % 2
        i0, i1 = half * NO, (half + 1) * NO
        # roll the sender axis so the own receiver half is columns [0, NO)
        xb = cc(np.roll(x[:, b, :].T, -i0, axis=1))
        sb = cc(np.roll(send[:, b, :].T, -i0, axis=1))
        rsb = cc(np.roll(res_s[:, b, :].T, -i0, axis=1))
        mrow = np.roll(mask[b, 0, i0:i1, :], -i0, axis=1)
        m = {
            "xT": xb, "sendT": sb, "res_sT": rsb,
            "recvTo": cc(recv[i0:i1, b, :].T),
            "res_rTo": cc(res_r[i0:i1, b, :].T),
            "mask": cc((mrow.astype(f32) * -MASKM).astype(ml_dtypes.float8_e5m2)),
        }
        m.update(weights)
        in_maps.append(m)
    return in_maps


def kernel(**inputs):
    if "nc" not in _CACHE:
        _CACHE["nc"] = _build_program()
    nc = _CACHE["nc"]
    in_maps = _host_prep(inputs)
    res = run_bass_kernel_spmd(nc, in_maps, core_ids=list(range(8)))
    out = np.zeros((N, B, C), np.float32)
    vr2 = np.zeros((N, B, C), np.float32)
    vs2 = np.zeros((N, B, C), np.float32)
    for core in range(8):
        b, half = core // 2, core % 2
        i0, i1 = half * NO, (half + 1) * NO
        r_ = res.results[core]
        out[i0:i1, b, :] = r_["outT"].T
        vr2[i0:i1, b, :] = r_["vr2T"].T
        vs2[i0:i1, b, :] = r_["vs2T"].T
    return out, vr2, vs2


# revision 43
# speedup vs baseline: 3641.3670x; 1.0080x over previous
"""Trainium2 Bass kernel for nn_AttentionTIE (TIE-style edge-LayerNorm attention).

Sharding: 8 cores = (batch b = core//2) x (receiver-row half = core%2).
Each core computes full v_sender for its batch, attention for its 1536
receiver rows, and the three projected outputs for those rows.

v2 design (vs the f32 baseline):
- All big matmuls bf16 / f32r (1 cyc/row) instead of f32 (4 cyc/row).
- Centered values c_s = v_s - m_s, c_r = v_r - m_r kill the mean rank-2
  terms: std2 = u_i + w_j + (2/D) c_r.c_s, score = q.c_s + alpha_i,
  out_pre = (PV[c_s] + A*c_r) / denom. PV aug channels = [c_s | 1].
- ACT engine runs ONLY exp-table functions (Exp/Copy/Identity/Square):
  zero activation-table reloads in steady state (baseline lost 124us/core
  to Sqrt<->Exp table thrash).
- rsqrt via one custom DVE op (quadratic minimax seed + folded Newton
  step, emitted scaled by 2/(3*sqrt(3)) to fit the 8-stage DVE pipeline;
  the 3*sqrt(3)/2 constant folds into host-side q / proj weights).
- mask host-converted to fp8e5m2 {0, -128}, injected into the score PSUM
  via an fp8 identity matmul (same DMA bytes as u8, kills the 53us gpsimd
  u8->bf16 conversion).
- PT = P*Tq runs on the otherwise-idle Pool (gpsimd) engine.
- Host rolls x/send/res_s/mask columns so the own receiver half is always
  columns [0, NO) -- one static program for all 8 cores.
"""
import sys
from contextlib import ExitStack

import numpy as np

sys.path.insert(0, "/opt/trn_rl_repo")

import ml_dtypes  # noqa: E402
import concourse.bass as bass  # noqa: E402
import concourse.tile as tile  # noqa: E402
from concourse import bacc  # noqa: E402
from concourse import mybir  # noqa: E402
from concourse.bass_utils import run_bass_kernel_spmd  # noqa: E402

# ---------------------------------------------------------------- custom DVE op
# Tq = s - x*s^3 with s = C0 + x*(C1 + x*C2): equals (2/(3*sqrt(3)))*rsqrt(x)
# for the minimax coefficients below (x in [0.85, 4.6], max rel err 2.6e-3
# after the folded Newton step). Exactly 8 ALU stages == the v3 pipeline.
import concourse.dve_ops as dve_ops  # noqa: E402
from concourse.dve_spec import C0, C1, C2, Spec, Src0, sq  # noqa: E402

RSQ_NAME = "RSQRT_SCALED_ANT"
RSQ_C = (0.7774802208033782, -0.23189214904335725, 0.02691905161330013)
RSQ_K = 3.0 * np.sqrt(3.0)  # consumers multiply this back in (host-folded)


def _rsq_ref(in0, in1, s0, s1, imm2):
    s = s0 + in0 * (s1 + in0 * imm2)
    return (s - in0 * s * s * s).astype(np.float32)


def _register_rsqrt_op():
    for op in dve_ops.OPS:
        if op.name == RSQ_NAME:
            return op
    _x = Src0
    _s = C0 + _x * (C1 + _x * C2)
    spec = Spec(body=_s - _x * (_s * sq(_s)), reference=_rsq_ref)
    row = dve_ops._CUSTOM_DVE_ROW_BASE + len(dve_ops.OPS)
    assert row < 0x20
    from concourse.dve_spec import lower as _lower
    from concourse.dve_uop import DveOpSpec

    shas = {}
    for ver in ("v3", "v4"):
        s = DveOpSpec(
            name=RSQ_NAME, opcode=row, uops=_lower(spec, ver=ver), rd1_en=False
        )
        shas[ver] = s.sha(ver)
    op = dve_ops.DveOp(RSQ_NAME, spec, subdim=False, uops_sha=shas)
    dve_ops.OPS.append(op)
    dve_ops.CUSTOM_DVE_SPECS[RSQ_NAME] = spec
    dve_ops._SUB_OPCODE_FOR_NAME[RSQ_NAME] = row
    return op


RSQ_OP = _register_rsqrt_op()

# ---------------------------------------------------------------------- consts
N, B, C = 3072, 4, 128
NO = N // 2          # own receiver rows per core
ITI = NO // 128      # 12 i-tiles
JCH = N // 512       # 6 j-chunks
JT = N // 128        # 24 j-tiles
EPS = 1e-5
SCALE = C ** -0.5
MASKM = 128.0        # fp8e5m2-exact mask bias (pre-Tq units)

F32 = mybir.dt.float32
F32R = mybir.dt.float32r
BF16 = mybir.dt.bfloat16
FP8 = mybir.dt.float8e5
AF = mybir.ActivationFunctionType
ALU = mybir.AluOpType
AX = mybir.AxisListType

_CACHE = {}


def _build_program():
    nc = bacc.Bacc("TRN2", target_bir_lowering=False, debug=False, num_devices=8)

    def din(name, shape, dtype=F32):
        return nc.dram_tensor(name, list(shape), dtype, kind="ExternalInput").ap()

    def dout(name, shape, dtype=F32):
        return nc.dram_tensor(name, list(shape), dtype, kind="ExternalOutput").ap()

    xT_d = din("xT", [C, N], F32R)
    sendT_d = din("sendT", [C, N], F32R)
    res_sT_d = din("res_sT", [C, N], F32R)
    recvTo_d = din("recvTo", [C, NO], F32R)
    res_rTo_d = din("res_rTo", [C, NO], F32R)
    mask_d = din("mask", [NO, N], FP8)
    w_send_d = din("w_send", [C, C], F32R)
    w_mem_d = din("w_mem", [C, C], F32R)
    w_recv_d = din("w_recv", [C, C], F32R)
    w_qs_d = din("w_qs", [C, C], F32R)
    w_proj_d = din("w_proj", [C, C], F32R)
    w_r_d = din("w_r", [C, C], F32R)
    w_s_d = din("w_s", [C, C], F32R)
    bp_d = din("bp", [C, 1])
    br_d = din("br", [C, 1])
    bs_d = din("bs", [C, 1])
    idf_d = din("idf", [C, C])
    idb_d = din("idb", [C, C], BF16)
    idr_d = din("idr", [C, C], F32R)
    oneD_d = din("oneD", [C, 1], F32R)
    one_d = din("one", [C, 1], F32R)
    onesr_d = din("onesr", [1, C], F32R)
    onesN_d = din("onesN", [1, N], F32R)
    id8_d = din("id8", [C, C], FP8)

    scr_ms_d = nc.dram_tensor("scr_ms", [1, N], F32).ap()
    scr_msq_d = nc.dram_tensor("scr_msq", [1, N], F32).ap()
    scr_w_d = nc.dram_tensor("scr_w", [1, N], F32).ap()
    scr_mr_d = nc.dram_tensor("scr_mr", [1, NO], F32).ap()
    scr_mrq_d = nc.dram_tensor("scr_mrq", [1, NO], F32).ap()
    scr_sq_d = nc.dram_tensor("scr_sq", [1, NO], F32).ap()
    scr_qv_d = nc.dram_tensor("scr_qv", [1, NO], F32).ap()
    scr_u_d = nc.dram_tensor("scr_u", [1, NO], F32).ap()
    outT_d = dout("outT", [C, NO])
    vr2T_d = dout("vr2T", [C, NO])
    vs2T_d = dout("vs2T", [C, NO])

    with tile.TileContext(nc) as tc, ExitStack() as ctx:
        const = ctx.enter_context(tc.tile_pool(name="const", bufs=1))
        per = ctx.enter_context(tc.tile_pool(name="per", bufs=1))
        stat = ctx.enter_context(tc.tile_pool(name="stat", bufs=1))
        stmp = ctx.enter_context(tc.tile_pool(name="stmp", bufs=2))
        ck = ctx.enter_context(tc.tile_pool(name="ck", bufs=7))
        strm = ctx.enter_context(tc.tile_pool(name="strm", bufs=6))
        mpool = ctx.enter_context(tc.tile_pool(name="mask", bufs=2))
        ps_mm = ctx.enter_context(tc.tile_pool(name="ps_mm", bufs=5, space="PSUM"))
        ps_pv = ctx.enter_context(tc.tile_pool(name="ps_pv", bufs=2, space="PSUM"))

        # ---------------- constants ----------------
        def cload(name, d_ap, shape, dtype=F32):
            t = const.tile(shape, dtype, tag=name)
            nc.sync.dma_start(t[:], d_ap)
            return t

        w_send = cload("w_send", w_send_d, [C, C], F32R)
        w_mem = cload("w_mem", w_mem_d, [C, C], F32R)
        w_recv = cload("w_recv", w_recv_d, [C, C], F32R)
        w_qs = cload("w_qs", w_qs_d, [C, C], F32R)
        w_proj = cload("w_proj", w_proj_d, [C, C], F32R)
        w_r = cload("w_r", w_r_d, [C, C], F32R)
        w_s = cload("w_s", w_s_d, [C, C], F32R)
        bp = cload("bp", bp_d, [C, 1])
        br_c = cload("br_c", br_d, [C, 1])
        bs_c = cload("bs_c", bs_d, [C, 1])
        idf = cload("idf", idf_d, [C, C])
        idb = cload("idb", idb_d, [C, C], BF16)
        id8 = cload("id8", id8_d, [C, C], FP8)
        idr = cload("idr", idr_d, [C, C], F32R)

        oneD = cload("oneD", oneD_d, [C, 1], F32R)      # 1/C col (means)
        one = cload("one", one_d, [C, 1], F32R)         # ones col (sums)
        ones_row = cload("ones_row", onesr_d, [1, C], F32R)  # bcast lhsT

        def r(ap):
            return ap.bitcast(F32R)

        # persistent tensors
        v_sT = per.tile([C, N], F32R)
        v_rT = per.tile([C, NO], F32R)
        qT = per.tile([C, NO], F32R)
        qTb = per.tile([C, NO], BF16)
        c_sT = per.tile([C, N], BF16)
        c_rTs = per.tile([C, NO], BF16)         # (v_r - m_r) * 2/C
        c_r_nat = per.tile([C, ITI * C], BF16)  # scaled c_r, natural layout
        c_s_aug = per.tile([C, JT * (C + 1)], BF16)
        outT_pre = per.tile([C, NO], F32R)

        aug_lhsT = stat.tile([2, NO], F32R)      # row0 = u_i+eps, row1 = 1
        aug_rhs = stat.tile([2, N], F32R)        # row0 = 1, row1 = w_j
        m_s_row = stat.tile([1, N], F32R)
        msq_row = stat.tile([1, N], F32)
        m_r_row = stat.tile([1, NO], F32R)
        mrq_row = stat.tile([1, NO], F32)
        sumq_row = stat.tile([1, NO], F32)
        qvr_row = stat.tile([1, NO], F32)
        alpha_cols = stat.tile([C, ITI], F32)
        m_s_cols = stat.tile([C, JT], F32)
        neg_ms_cols = stat.tile([C, JT], F32)
        msq_cols = stat.tile([C, JT], F32)
        w_cols = stat.tile([C, JT], F32R)
        m_r_cols = stat.tile([C, ITI], F32)
        mrq_cols = stat.tile([C, ITI], F32)
        sumq_cols = stat.tile([C, ITI], F32)
        u_cols = stat.tile([C, ITI], F32R)

        nc.sync.dma_start(aug_lhsT[1:2, :], onesN_d[:, 0:NO])
        nc.sync.dma_start(aug_rhs[0:1, :], onesN_d)

        def stream(d_ap, sl, tag="instream"):
            t = strm.tile([C, 512], F32R, tag=tag)
            nc.sync.dma_start(t[:], d_ap[:, sl])
            return t

        # -------- phase 1: value tensors --------
        # residuals folded into the PSUM accumulation via f32r identity
        # matmul; PSUM evacuated by ACT (keeps DVE free for the main loop).
        for jc in range(JCH):
            sl = bass.ts(jc, 512)
            xc = stream(xT_d, sl)
            sc = stream(sendT_d, sl)
            rc = stream(res_sT_d, sl)
            ps = ps_mm.tile([C, 512], F32, tag="mm")
            nc.tensor.matmul(ps[:], w_send[:], xc[:], start=True, stop=False)
            nc.tensor.matmul(ps[:], w_mem[:], sc[:], start=False, stop=False)
            nc.tensor.matmul(ps[:], idr[:], rc[:], start=False, stop=True)
            nc.scalar.copy(v_sT[:, sl], ps[:])
            # m_s and mean-square rank-1s straight off the SBUF copy
            psm_f = ps_mm.tile([C, 512], F32, tag="mm")
            nc.tensor.matmul(psm_f[0:1, :], oneD[:], v_sT[:, sl], start=True, stop=True)
            nc.vector.tensor_copy(out=m_s_row[:, sl], in_=psm_f[0:1, :])
            sqc = ck.tile([C, 512], F32R, tag="sqc")
            nc.scalar.activation(sqc[:], v_sT[:, sl], AF.Square)
            psq_f = ps_mm.tile([C, 512], F32, tag="mm")
            nc.tensor.matmul(psq_f[0:1, :], oneD[:], sqc[:], start=True, stop=True)
            nc.vector.tensor_copy(out=msq_row[:, sl], in_=psq_f[0:1, :])
            # c_sT = v_sT - bcast(m_s)
            psb = ps_mm.tile([C, 512], F32, tag="mm")
            nc.tensor.matmul(psb[:], ones_row[:], m_s_row[0:1, sl], start=True, stop=True)
            nc.vector.tensor_tensor(out=c_sT[:, sl], in0=v_sT[:, sl], in1=psb[:], op=ALU.subtract)

        for c3 in range(NO // 512):
            sl = bass.ts(c3, 512)
            xc = stream(xT_d, sl)
            rcv = stream(recvTo_d, sl)
            rrc = stream(res_rTo_d, sl)
            ps2 = ps_mm.tile([C, 512], F32, tag="mm")
            nc.tensor.matmul(ps2[:], w_recv[:], xc[:], start=True, stop=False)
            nc.tensor.matmul(ps2[:], w_mem[:], rcv[:], start=False, stop=False)
            nc.tensor.matmul(ps2[:], idr[:], rrc[:], start=False, stop=True)
            nc.scalar.copy(v_rT[:, sl], ps2[:])
            ps3 = ps_mm.tile([C, 512], F32, tag="mm")
            nc.tensor.matmul(ps3[:], w_qs[:], xc[:], start=True, stop=True)
            nc.scalar.copy(qT[:, sl], ps3[:])
            nc.scalar.copy(qTb[:, sl], ps3[:])

        # -------- phase 2: stats (row extraction, arithmetic in col space) --
        for c3 in range(NO // 512):
            sl = bass.ts(c3, 512)
            psm_f = ps_mm.tile([C, 512], F32, tag="mm")
            nc.tensor.matmul(psm_f[0:1, :], oneD[:], v_rT[:, sl], start=True, stop=True)
            nc.vector.tensor_copy(out=m_r_row[:, sl], in_=psm_f[0:1, :])
            sqc = ck.tile([C, 512], F32R, tag="sqc")
            nc.scalar.activation(sqc[:], v_rT[:, sl], AF.Square)
            psq_f = ps_mm.tile([C, 512], F32, tag="mm")
            nc.tensor.matmul(psq_f[0:1, :], oneD[:], sqc[:], start=True, stop=True)
            nc.vector.tensor_copy(out=mrq_row[:, sl], in_=psq_f[0:1, :])
            pss_f = ps_mm.tile([C, 512], F32, tag="mm")
            nc.tensor.matmul(pss_f[0:1, :], one[:], qT[:, sl], start=True, stop=True)
            nc.vector.tensor_copy(out=sumq_row[:, sl], in_=pss_f[0:1, :])
            qv = ck.tile([C, 512], F32R, tag="sqc")
            nc.vector.tensor_tensor(out=qv[:], in0=qT[:, sl], in1=v_rT[:, sl], op=ALU.mult)
            psa_f = ps_mm.tile([C, 512], F32, tag="mm")
            nc.tensor.matmul(psa_f[0:1, :], one[:], qv[:], start=True, stop=True)
            nc.vector.tensor_copy(out=qvr_row[:, sl], in_=psa_f[0:1, :])
            # c_rTs = (v_r - m_r) * 2/C  -> bf16
            psb = ps_mm.tile([C, 512], F32, tag="mm")
            nc.tensor.matmul(psb[:], ones_row[:], m_r_row[0:1, sl], start=True, stop=True)
            nc.vector.tensor_tensor(out=c_rTs[:, sl], in0=v_rT[:, sl], in1=psb[:], op=ALU.subtract)
        nc.scalar.mul(c_rTs[:], c_rTs[:], 2.0 / C)

        # rows -> column layouts via direct SBUF->SBUF strided DMA
        def to_cols(row_t, scr_d, cols_t):
            src_ap = row_t[:]
            if src_ap.dtype != scr_d.dtype:
                src_ap = src_ap.bitcast(scr_d.dtype)
            nc.sync.dma_start(scr_d, src_ap)
            nc.sync.dma_start(
                cols_t[:], scr_d.rearrange("o (t p) -> (o p) t", p=128))

        to_cols(m_s_row, scr_ms_d, m_s_cols)
        to_cols(msq_row, scr_msq_d, msq_cols)
        to_cols(m_r_row, scr_mr_d, m_r_cols)
        to_cols(mrq_row, scr_mrq_d, mrq_cols)
        to_cols(sumq_row, scr_sq_d, sumq_cols)
        to_cols(qvr_row, scr_qv_d, alpha_cols)  # raw q.v_r; corrected below

        nc.scalar.mul(neg_ms_cols[:], m_s_cols[:], -1.0)
        # w = msq - m_s^2  (per j-tile columns)
        nc.vector.tensor_tensor(out=w_cols[:], in0=m_s_cols[:], in1=m_s_cols[:], op=ALU.mult)
        nc.vector.tensor_tensor(out=w_cols[:], in0=msq_cols[:], in1=w_cols[:], op=ALU.subtract)
        # u_eps = mrq - m_r^2 + EPS
        nc.vector.tensor_tensor(out=u_cols[:], in0=m_r_cols[:], in1=m_r_cols[:], op=ALU.mult)
        nc.vector.scalar_tensor_tensor(
            out=u_cols[:], in0=mrq_cols[:], scalar=EPS, in1=u_cols[:],
            op0=ALU.add, op1=ALU.subtract)
        # alpha = q.v_r - sumq*m_r
        nc.vector.tensor_tensor(out=sumq_cols[:], in0=sumq_cols[:], in1=m_r_cols[:], op=ALU.mult)
        nc.vector.tensor_tensor(out=alpha_cols[:], in0=alpha_cols[:], in1=sumq_cols[:], op=ALU.subtract)

        # w and u_eps back to row layout for the aug matmul operands
        nc.sync.dma_start(scr_w_d.rearrange("o (t p) -> (o p) t", p=128),
                          w_cols[:].bitcast(F32))
        nc.sync.dma_start(aug_rhs[1:2, :], scr_w_d.bitcast(F32R))
        nc.sync.dma_start(scr_u_d.rearrange("o (t p) -> (o p) t", p=128),
                          u_cols[:].bitcast(F32))
        nc.sync.dma_start(aug_lhsT[0:1, :], scr_u_d.bitcast(F32R))

        # c_s natural + ones channel; center during PSUM evacuation with
        # per-partition bias = -m_s for the j-tile.
        c_s_aug_r = c_s_aug[:].rearrange("p (t c) -> p t c", c=C + 1)
        for g in range(JT // 4):
            pst = ps_mm.tile([C, 512], F32, tag="mm")
            for t in range(4):
                jt = g * 4 + t
                nc.tensor.transpose(pst[:, bass.ts(t, 128)], v_sT[:, bass.ts(jt, 128)].bitcast(F32), idf[:])
            for t in range(4):
                jt = g * 4 + t
                nc.scalar.activation(
                    c_s_aug_r[:, jt, 0:C], pst[:, bass.ts(t, 128)], AF.Identity,
                    bias=neg_ms_cols[:, jt:jt + 1])
        nc.gpsimd.memset(c_s_aug_r[:, :, C:C + 1], 1.0)

        # c_r natural (scaled): transpose c_rTs (bf16, 1 cyc/row)
        neg_mr2_cols = stat.tile([C, ITI], F32)
        nc.scalar.mul(neg_mr2_cols[:], m_r_cols[:], -2.0 / C)
        for g in range(ITI // 4):
            pst = ps_mm.tile([C, 512], F32, tag="mm")
            for t in range(4):
                it = g * 4 + t
                nc.tensor.transpose(pst[:, bass.ts(t, 128)], v_rT[:, bass.ts(it, 128)].bitcast(F32), idf[:])
            for t in range(4):
                it = g * 4 + t
                nc.scalar.activation(
                    c_r_nat[:, bass.ts(it, 128)], pst[:, bass.ts(t, 128)], AF.Identity,
                    bias=neg_mr2_cols[:, it:it + 1], scale=2.0 / C)

        # vr2 / vs2 projections don't depend on attention: emit them here so
        # they overlap the main loop (outT projection stays at the end).
        for w, bias_col, rhs_t, out_d in (
            (w_r, br_c, v_rT, vr2T_d),
            (w_s, bs_c, v_sT, vs2T_d),
        ):
            for c3 in range(NO // 512):
                sl = bass.ts(c3, 512)
                ps = ps_mm.tile([C, 512], F32, tag="mm")
                nc.tensor.matmul(ps[:], w[:], rhs_t[:, sl], start=True, stop=True)
                ob = stmp.tile([C, 512], F32, tag="ob")
                nc.scalar.activation(ob[:], ps[:], AF.Identity, bias=bias_col[:])
                nc.sync.dma_start(out_d[:, sl], ob[:])

        # -------- phase 3: main attention loop (flat software pipeline) ----
        # Per slot (it, jc): emit MM + elementwise chain; the transpose + PV
        # stage for slot k-LAG runs behind so the PE's in-order queue never
        # head-of-line blocks on the elementwise chain. PT transposes go
        # through the DMA XBAR (idle engine) instead of PE+ACT evacuation.
        rsq = RSQ_C
        LAG = 4
        EPI = 3
        slots = [(it, jc) for it in range(ITI) for jc in range(JCH)]
        mk8_tiles = {}
        den_parts = {}
        pv_tiles = {}
        pv_sbs = {}
        ptc_tiles = {}

        def prefetch_mask(it):
            mk8 = mpool.tile([C, N], FP8, tag="mk8")
            nc.sync.dma_start(mk8[:], mask_d[bass.ts(it, 128), :])
            mk8_tiles[it] = mk8

        def emit_front(it, jc):
            isl = bass.ts(it, 128)
            jsl = bass.ts(jc, 512)
            if jc == 0:
                den_parts[it] = stmp.tile([C, 8], F32, tag="den_part", name=f"denp{it}")
                pv_tiles[it] = ps_pv.tile([C, C + 1], F32, tag="pv", name=f"pv{it}")
                if it + 1 < ITI:
                    prefetch_mask(it + 1)
            ps_v = ps_mm.tile([C, 512], F32, tag="mm")
            nc.tensor.matmul(ps_v[:], c_rTs[:, isl], c_sT[:, jsl], start=True, stop=False)
            nc.tensor.matmul(ps_v[:], aug_lhsT[:, isl], aug_rhs[:, jsl], start=False, stop=True)
            ps_s = ps_mm.tile([C, 512], F32, tag="mm")
            nc.tensor.matmul(ps_s[:], qTb[:, isl], c_sT[:, jsl], start=True, stop=False)
            nc.tensor.matmul(ps_s[:], id8[:], mk8_tiles[it][:, jsl], start=False, stop=True)

            tq = ck.tile([C, 512], F32, tag="tq")
            nc.vector._custom_dve(
                RSQ_OP, out=tq[:], in0=ps_v[:],
                s0=rsq[0], s1=rsq[1], imm2=rsq[2])
            uc = ck.tile([C, 512], BF16, tag="uc")
            nc.vector.scalar_tensor_tensor(
                out=uc[:], in0=ps_s[:], scalar=alpha_cols[:, it:it + 1],
                in1=tq[:], op0=ALU.add, op1=ALU.mult)
            pc = ck.tile([C, 512], BF16, tag="pc")
            nc.scalar.activation(pc[:], uc[:], AF.Exp, accum_out=den_parts[it][:, jc:jc + 1])
            ptc = ck.tile([C, 512], BF16, tag="ptc")
            nc.gpsimd.tensor_tensor(out=ptc[:], in0=pc[:], in1=tq[:], op=ALU.mult)
            ptc_tiles[(it, jc)] = ptc

        def emit_back(it, jc):
            ptc = ptc_tiles.pop((it, jc))
            ptt = ck.tile([C, 512], BF16, tag="ptt")
            nc.sync.dma_start_transpose(
                ptt[:].rearrange("p (t c) -> p t c", c=128), ptc[:])
            pv = pv_tiles[it]
            for t in range(4):
                jt = jc * 4 + t
                nc.tensor.matmul(
                    pv[:], ptt[:, bass.ts(t, 128)], c_s_aug_r[:, jt, :],
                    start=(jc == 0 and t == 0), stop=(jc == JCH - 1 and t == 3))


        def emit_epilogue(it):
            isl = bass.ts(it, 128)
            pv_sb = pv_tiles.pop(it)
            den_part = den_parts.pop(it)
            den = stmp.tile([C, 1], F32, tag="den")
            nc.vector.tensor_reduce(den[:], den_part[:, 0:JCH], axis=AX.X, op=ALU.add)
            rcol = stmp.tile([C, 1], F32, tag="rcol")
            nc.vector.reciprocal_approx_fast(out=rcol[:], in_=den[:])
            a2 = stmp.tile([C, 1], F32, tag="a2")
            nc.scalar.mul(a2[:], pv_sb[:, C:C + 1], float(C) / 2.0)
            x1 = stmp.tile([C, C], F32, tag="x1")
            nc.vector.scalar_tensor_tensor(
                out=x1[:], in0=c_r_nat[:, isl], scalar=a2[:], in1=pv_sb[:, 0:C],
                op0=ALU.mult, op1=ALU.add)
            x2 = stmp.tile([C, C], F32, tag="x2")
            nc.scalar.mul(x2[:], x1[:], rcol[:])
            pso = ps_mm.tile([C, 512], F32, tag="mm")
            nc.tensor.transpose(pso[:, 0:C], x2[:], idf[:])
            nc.scalar.copy(outT_pre[:, isl], pso[:, 0:C])

        prefetch_mask(0)
        for k in range(len(slots) + LAG + EPI):
            if k < len(slots):
                emit_front(*slots[k])
            kb = k - LAG
            if 0 <= kb < len(slots):
                emit_back(*slots[kb])
            ke = k - LAG - EPI
            if ke >= 0 and slots[ke][1] == JCH - 1:
                emit_epilogue(slots[ke][0])

        # -------- phase 4: attention output projection --------
        for c3 in range(NO // 512):
            sl = bass.ts(c3, 512)
            ps = ps_mm.tile([C, 512], F32, tag="mm")
            nc.tensor.matmul(ps[:], w_proj[:], outT_pre[:, sl], start=True, stop=True)
            ob = stmp.tile([C, 512], F32, tag="ob")
            nc.scalar.activation(ob[:], ps[:], AF.Identity, bias=bp[:])
            nc.sync.dma_start(outT_d[:, sl], ob[:])

    nc.compile()
    return nc


def _host_prep(inputs):
    """Returns in_maps list of 8 dicts."""
    f32 = np.float32
    x = np.ascontiguousarray(np.asarray(inputs["x"], f32))
    recv = np.asarray(inputs["receiver_val_res"], f32)
    send = np.asarray(inputs["sender_val_res"], f32)
    res_r = np.asarray(inputs["residual_receiver"], f32)
    res_s = np.asarray(inputs["residual_sender"], f32)
    mask = np.asarray(inputs["attn_mask"])
    ra = np.asarray(inputs["relation_attn"], f32)
    q_w = np.asarray(inputs["q_w"], f32)
    proj_w = np.asarray(inputs["proj_w"], f32)
    proj_b = np.asarray(inputs["proj_b"], f32)
    r_w = np.asarray(inputs["r_w"], f32)
    r_b = np.asarray(inputs["r_b"], f32)
    s_w = np.asarray(inputs["s_w"], f32)
    s_b = np.asarray(inputs["s_b"], f32)
    n_weight = np.asarray(inputs["n_weight"], f32)
    n_bias = np.asarray(inputs["n_bias"], f32)

    mem_w, recv_w, send_w = ra[:, :C], ra[:, C:2 * C], ra[:, 2 * C:]
    w_proj_eff = proj_w * n_weight[None, :] * (RSQ_K / 2.0)
    b_proj_eff = proj_w @ n_bias + proj_b
    q_scale = SCALE * RSQ_K / 2.0

    cc = np.ascontiguousarray
    weights = {
        "w_send": cc(send_w.T), "w_mem": cc(mem_w.T), "w_recv": cc(recv_w.T),
        "w_qs": cc(q_w.T * q_scale),
        "w_proj": cc(w_proj_eff.T),
        "w_r": cc(r_w.T), "w_s": cc(s_w.T),
        "bp": cc(b_proj_eff[:, None]), "br": cc(r_b[:, None]), "bs": cc(s_b[:, None]),
        "idf": cc(np.eye(C, dtype=f32)),
        "idr": cc(np.eye(C, dtype=f32)),
        "oneD": cc(np.full((C, 1), 1.0 / C, f32)),
        "one": cc(np.ones((C, 1), f32)),
        "onesr": cc(np.ones((1, C), f32)),
        "onesN": cc(np.ones((1, N), f32)),
        "idb": cc(np.eye(C).astype(ml_dtypes.bfloat16)),
        "id8": cc(np.eye(C).astype(ml_dtypes.float8_e5m2)),
    }

    in_maps = []
    for core in range(8):
        b, half = core // 2, core % 2
        i0, i1 = half * NO, (half + 1) * NO
        # roll the sender axis so the own receiver half is columns [0, NO)
        xb = cc(np.roll(x[:, b, :].T, -i0, axis=1))
        sb = cc(np.roll(send[:, b, :].T, -i0, axis=1))
        rsb = cc(np.roll(res_s[:, b, :].T, -i0, axis=1))
        mrow = np.roll(mask[b, 0, i0:i1, :], -i0, axis=1)
        m = {
            "xT": xb, "sendT": sb, "res_sT": rsb,
            "recvTo": cc(recv[i0:i1, b, :].T),
            "res_rTo": cc(res_r[i0:i1, b, :].T),
            "mask": cc((mrow.astype(f32) * -MASKM).astype(ml_dtypes.float8_e5m2)),
        }
        m.update(weights)
        in_maps.append(m)
    return in_maps


def kernel(**inputs):
    if "nc" not in _CACHE:
        _CACHE["nc"] = _build_program()
    nc = _CACHE["nc"]
    in_maps = _host_prep(inputs)
    res = run_bass_kernel_spmd(nc, in_maps, core_ids=list(range(8)))
    out = np.zeros((N, B, C), np.float32)
    vr2 = np.zeros((N, B, C), np.float32)
    vs2 = np.zeros((N, B, C), np.float32)
    for core in range(8):
        b, half = core // 2, core % 2
        i0, i1 = half * NO, (half + 1) * NO
        r_ = res.results[core]
        out[i0:i1, b, :] = r_["outT"].T
        vr2[i0:i1, b, :] = r_["vr2T"].T
        vs2[i0:i1, b, :] = r_["vs2T"].T
    return out, vr2, vs2
